# revision 11
# baseline (speedup 1.0000x reference)
"""Trainium2 Bass kernel for nn_EncoderLayer (B=8, S=1024, D=1024, H=16, FF=2048).

Sharding: data-parallel over batch — core i handles batch element i. No
collectives. GEMMs run in bf16 (1 cyc/row on PE); residual stream f32.

Per-core dataflow (S=seq, D=feat; P=128 partitions):
  P1  LN1 apply (host-computed mean/rstd) -> x2j bf16, DMA-transpose -> x2t
  V   vaug [S, H, 65] bf16 (65th column = ones; softmax denominator for free)
  P3  per head-pair pr: QT/KT = w^T @ x2t (SBUF-resident, bf16),
      per j: scoresT = K @ Q^T (row-packed 64x2 heads), exp via ACT with
      per-partition mask bias, attnT[65, S] += [V|1]^T @ expT.
      Denominators: PE-transpose the two [1,S] rows -> [128,16], one cheap
      DVE reciprocal, DMA round-trip to broadcast [64,S], normalize on evac.
  P4  out-proj O = catT^T @ wo (+ residual x) -> out1 seq-major (f32)
  P5  LN2 (bn_stats) -> x2b bf16, DMA-transpose -> x2t (reused)
  P6  HT = w1^T @ x2bt, relu (+b1) -> ht bf16
  P7  out2 = ht^T @ w2 (+ out1 residual) -> y
"""
import sys

sys.path.insert(0, "/opt/trn_rl_repo")

import numpy as np
import ml_dtypes

import concourse.bass as bass
import concourse.mybir as mybir
from concourse import bacc
from concourse.tile import TileContext
from concourse.bass_utils import run_bass_kernel_spmd
from concourse.masks import make_identity

P = 128
S = 1024
D = 1024
H = 16
DK = 64
F = 2048
NT = S // P   # seq tiles
KD = D // P   # feature k-tiles
KF = F // P   # ff k-tiles
EPS = 1e-6

F32 = mybir.dt.float32
BF16 = mybir.dt.bfloat16
Alu = mybir.AluOpType
Act = mybir.ActivationFunctionType
BF16NP = ml_dtypes.bfloat16

# smalls layout (columns of a [128, 56] tensor)
C_MU, C_R1, C_MB, C_BQ, C_BK, C_B1 = 0, 8, 16, 24, 32, 40  # b1 gets 16 cols

_CACHE = {}
LAST_RESULT = None


def _build(flags):
    has_bv, has_bo, has_b2 = flags
    nc = bacc.Bacc()

    x_d = nc.dram_tensor("x", [NT, P, D], F32, kind="ExternalInput")
    sm_d = nc.dram_tensor("smalls", [P, 56], F32, kind="ExternalInput")
    wq_d = nc.dram_tensor("wq", [KD, P, D], BF16, kind="ExternalInput")
    wk_d = nc.dram_tensor("wk", [KD, P, D], BF16, kind="ExternalInput")
    wv_d = nc.dram_tensor("wv", [KD, P, D], BF16, kind="ExternalInput")
    wo_d = nc.dram_tensor("wo", [KD, P, D], BF16, kind="ExternalInput")
    w1_d = nc.dram_tensor("w1", [KD, P, F], BF16, kind="ExternalInput")
    w2_d = nc.dram_tensor("w2", [KF, P, D], BF16, kind="ExternalInput")
    if has_bv:
        bv_d = nc.dram_tensor("bv", [1, D], F32, kind="ExternalInput")
    if has_bo:
        bo_d = nc.dram_tensor("bo", [1, D], F32, kind="ExternalInput")
    if has_b2:
        b2_d = nc.dram_tensor("b2", [1, D], F32, kind="ExternalInput")
    y_d = nc.dram_tensor("y", [NT, P, D], F32, kind="ExternalOutput")

    rd_d = nc.dram_tensor("rd_scratch", [H, S], F32)
    dn_d = nc.dram_tensor("dn_scratch", [H, S], F32)

    with TileContext(nc) as tc:
        with tc.tile_pool(name="const", bufs=1) as constp, \
             tc.tile_pool(name="big", bufs=1) as bigp:
            smalls = constp.tile([P, 56], F32)
            nc.sync.dma_start(out=smalls, in_=sm_d[:, :])

            def bias_bcast(dram_row):
                src_ap = dram_row[0:1, :]
                bc_ap = bass.AP(tensor=src_ap.tensor, offset=src_ap.offset,
                                ap=[[0, P]] + list(src_ap.ap)[1:])
                bc = constp.tile([P, D], F32)
                nc.sync.dma_start(out=bc, in_=bc_ap)
                return bc

            bvB = bias_bcast(bv_d) if has_bv else None
            boB = bias_bcast(bo_d) if has_bo else None
            b2B = bias_bcast(b2_d) if has_b2 else None

            out1 = bigp.tile([P, NT, D], F32, tag="out1")
            x2t = bigp.tile([P, KD, S], BF16, tag="x2t")

            attl_cm = tc.tile_pool(name="attl", bufs=1)
            attl = attl_cm.__enter__()
            vaug = attl.tile([P, NT, H, 65], BF16, tag="vaug")
            cat = attl.tile([P, KD, S], BF16, tag="cat")

            # ---------------- P1: LN1 apply + DMA transpose ----------------
            with tc.tile_pool(name="p1", bufs=3) as p1:
                for j in range(NT):
                    xj = p1.tile([P, D], F32, tag="xj")
                    nc.sync.dma_start(out=xj, in_=x_d[j])
                    x2j = p1.tile([P, D], BF16, tag="x2j")
                    nc.vector.tensor_scalar(
                        out=x2j, in0=xj,
                        scalar1=smalls[:, C_MU + j:C_MU + j + 1],
                        scalar2=smalls[:, C_R1 + j:C_R1 + j + 1],
                        op0=Alu.subtract, op1=Alu.mult)
                    nc.sync.dma_start(
                        out=x2t[:, :, j * P:(j + 1) * P], in_=x2j, transpose=True)

            # ---------------- V: vaug [S, H, 65] ----------------
            with tc.tile_pool(name="wvp", bufs=1) as wvp, \
                 tc.tile_pool(name="psv", bufs=4, space="PSUM") as psv:
                ones16 = constp.tile([P, H], F32)
                nc.vector.memset(ones16, 1.0)
                for i in range(NT):
                    nc.vector.tensor_copy(
                        out=vaug[:, i, :, 64:65],
                        in_=ones16.rearrange("p (h o) -> p h o", o=1))
                wv_sl = []
                for k in range(KD):
                    t = wvp.tile([P, D], BF16, tag=f"wv{k}")
                    nc.sync.dma_start(out=t, in_=wv_d[k])
                    wv_sl.append(t)
                for n in range(2):
                    for i in range(NT):
                        ps = psv.tile([P, 512], F32, tag="mm")
                        for k in range(KD):
                            nc.tensor.matmul(
                                ps, x2t[:, k, i * P:(i + 1) * P],
                                wv_sl[k][:, n * 512:(n + 1) * 512],
                                start=(k == 0), stop=(k == KD - 1))
                        dst = vaug[:, i, 8 * n:8 * n + 8, 0:64]
                        if has_bv:
                            nc.vector.tensor_add(
                                out=dst, in0=ps.rearrange("p (h c) -> p h c", c=64),
                                in1=bvB[:, n * 512:(n + 1) * 512].rearrange(
                                    "p (h c) -> p h c", c=64))
                        else:
                            nc.vector.tensor_copy(
                                out=dst, in_=ps.rearrange("p (h c) -> p h c", c=64))

            # -------- P3: per head-pair QT/KT + attention --------
            wopre_cm = tc.tile_pool(name="wopre", bufs=1)
            wopre = wopre_cm.__enter__()
            xr_cm = tc.tile_pool(name="xr", bufs=1)
            xrp = xr_cm.__enter__()
            wo_tiles = {}
            xm_sl = []
            with tc.tile_pool(name="wp", bufs=2) as wp, \
                 tc.tile_pool(name="qk", bufs=2) as qkp, \
                 tc.tile_pool(name="ep", bufs=3) as ep, \
                 tc.tile_pool(name="dn", bufs=2) as dnp, \
                 tc.tile_pool(name="rr", bufs=2) as rrp, \
                 tc.tile_pool(name="rb", bufs=2) as rbp, \
                 tc.tile_pool(name="sc", bufs=2, space="PSUM") as scp, \
                 tc.tile_pool(name="at", bufs=2, space="PSUM") as atp:
                for pr in range(KD):
                    hA, hB = 2 * pr, 2 * pr + 1
                    # --- QT / KT for this head pair (SBUF-resident bf16) ---
                    wq8 = wp.tile([P, KD, P], BF16, tag="wq8")
                    nc.sync.dma_start(
                        out=wq8,
                        in_=wq_d[:, :, pr * P:(pr + 1) * P].rearrange("k p m -> p k m"))
                    wk8 = wp.tile([P, KD, P], BF16, tag="wk8")
                    nc.sync.dma_start(
                        out=wk8,
                        in_=wk_d[:, :, pr * P:(pr + 1) * P].rearrange("k p m -> p k m"))
                    qps = atp.tile([P, S], F32, tag="at")
                    for n in range(2):
                        for k in range(KD):
                            nc.tensor.matmul(
                                qps[:, n * 512:(n + 1) * 512], wq8[:, k, :],
                                x2t[:, k, n * 512:(n + 1) * 512],
                                start=(k == 0), stop=(k == KD - 1))
                    qtp = qkp.tile([P, S], BF16, tag="qt")
                    nc.vector.tensor_scalar(
                        out=qtp, in0=qps,
                        scalar1=smalls[:, C_BQ + pr:C_BQ + pr + 1], scalar2=None,
                        op0=Alu.add)
                    kps = atp.tile([P, S], F32, tag="at")
                    for n in range(2):
                        for k in range(KD):
                            nc.tensor.matmul(
                                kps[:, n * 512:(n + 1) * 512], wk8[:, k, :],
                                x2t[:, k, n * 512:(n + 1) * 512],
                                start=(k == 0), stop=(k == KD - 1))
                    ktp = qkp.tile([P, S], BF16, tag="kt")
                    nc.vector.tensor_scalar(
                        out=ktp, in0=kps,
                        scalar1=smalls[:, C_BK + pr:C_BK + pr + 1], scalar2=None,
                        op0=Alu.add)

                    # --- attention over key tiles j ---
                    aA = atp.tile([P, S], F32, tag="at")
                    aB = atp.tile([P, S], F32, tag="at")
                    for j in range(NT):
                        scA = scp.tile([P, S], F32, tag="sc")
                        scB = scp.tile([P, S], F32, tag="sc")
                        for n in range(2):
                            nc.tensor.matmul(
                                scA[:, n * 512:(n + 1) * 512],
                                ktp[0:64, j * P:(j + 1) * P],
                                qtp[0:64, n * 512:(n + 1) * 512],
                                start=True, stop=True, tile_position=(0, 0))
                            nc.tensor.matmul(
                                scB[:, n * 512:(n + 1) * 512],
                                ktp[64:P, j * P:(j + 1) * P],
                                qtp[64:P, n * 512:(n + 1) * 512],
                                start=True, stop=True, tile_position=(64, 0))
                        eA = ep.tile([P, S], BF16, tag="exp")
                        nc.scalar.activation(
                            out=eA, in_=scA, func=Act.Exp,
                            bias=smalls[:, C_MB + j:C_MB + j + 1], scale=0.125)
                        eB = ep.tile([P, S], BF16, tag="exp")
                        nc.scalar.activation(
                            out=eB, in_=scB, func=Act.Exp,
                            bias=smalls[:, C_MB + j:C_MB + j + 1], scale=0.125)
                        for n in range(2):
                            nc.tensor.matmul(
                                aA[0:65, n * 512:(n + 1) * 512],
                                vaug[:, j, hA, :],
                                eA[:, n * 512:(n + 1) * 512],
                                start=(j == 0), stop=(j == NT - 1))
                            nc.tensor.matmul(
                                aB[0:65, n * 512:(n + 1) * 512],
                                vaug[:, j, hB, :],
                                eB[:, n * 512:(n + 1) * 512],
                                start=(j == 0), stop=(j == NT - 1))

                    # --- denominators: psum row -> DRAM -> strided gather
                    #     [p, a, h] -> one cheap recip -> DRAM -> bcast ---
                    dnA = dnp.tile([1, S], F32, tag="dnA")
                    nc.vector.tensor_copy(out=dnA, in_=aA[64:65, :])
                    nc.sync.dma_start(out=dn_d[hA:hA + 1, :], in_=dnA)
                    dnB = dnp.tile([1, S], F32, tag="dnB")
                    nc.vector.tensor_copy(out=dnB, in_=aB[64:65, :])
                    nc.sync.dma_start(out=dn_d[hB:hB + 1, :], in_=dnB)
                    rrs = rrp.tile([P, 2, NT], F32, tag="rrs")
                    for hb in (0, 1):
                        g_ap = bass.AP(tensor=dn_d[0:1, :].tensor,
                                       offset=(2 * pr + hb) * S,
                                       ap=[[1, P], [P, NT]])
                        nc.sync.dma_start(out=rrs[:, hb, :], in_=g_ap)
                    rrinv = rrp.tile([P, 2, NT], F32, tag="rr")
                    nc.vector.reciprocal(out=rrinv, in_=rrs)

                    def bcast_recip(hb):
                        h = 2 * pr + hb
                        dst = rd_d[h:h + 1, :].rearrange("o (a p) -> (o p) a", p=P)
                        nc.sync.dma_start(out=dst, in_=rrinv[:, hb, :])
                        s_ap = rd_d[h:h + 1, :]
                        bc_ap = bass.AP(tensor=s_ap.tensor, offset=s_ap.offset,
                                        ap=[[0, 64]] + list(s_ap.ap)[1:])
                        rbt = rbp.tile([64, S], F32, tag="rb")
                        nc.sync.dma_start(out=rbt, in_=bc_ap)
                        return rbt

                    rbA = bcast_recip(0)
                    nc.vector.tensor_mul(
                        out=cat[0:64, pr, :], in0=aA[0:64, :], in1=rbA)
                    rbB = bcast_recip(1)
                    stg = dnp.tile([64, S], BF16, tag="stg")
                    nc.vector.tensor_mul(out=stg, in0=aB[0:64, :], in1=rbB)
                    nc.sync.dma_start(out=cat[64:P, pr, :], in_=stg)

                    # prefetch wo and residual x while attention runs
                    if pr < 4:
                        for k in (2 * pr, 2 * pr + 1):
                            t = wopre.tile([P, D], BF16, tag=f"wo{k}", name=f"wo{k}")
                            nc.sync.dma_start(out=t, in_=wo_d[k])
                            wo_tiles[k] = t
                    else:
                        for m in (2 * (pr - 4), 2 * (pr - 4) + 1):
                            xm = xrp.tile([P, D], F32, tag=f"xm{m}", name=f"xm{m}")
                            nc.sync.dma_start(out=xm, in_=x_d[m])
                            xm_sl.append(xm)

            # ---------------- P4 + P5 merged m-loop ----------------
            with tc.tile_pool(name="p45", bufs=3) as p45, \
                 tc.tile_pool(name="pm", bufs=4, space="PSUM") as pm:
                for m in range(NT):
                    xm = xm_sl[m]
                    for n in range(2):
                        ps = pm.tile([P, 512], F32, tag="mm")
                        for k in range(KD):
                            nc.tensor.matmul(
                                ps, cat[:, k, m * P:(m + 1) * P],
                                wo_tiles[k][:, n * 512:(n + 1) * 512],
                                start=(k == 0), stop=(k == KD - 1))
                        dst = out1[:, m, n * 512:(n + 1) * 512]
                        nc.vector.tensor_add(
                            out=dst, in0=ps, in1=xm[:, n * 512:(n + 1) * 512])
                        if has_bo:
                            nc.vector.tensor_add(
                                out=dst, in0=dst, in1=boB[:, n * 512:(n + 1) * 512])
                    # LN2 for this m
                    row = out1[:, m, :]
                    st = p45.tile([P, 2, 6], F32, tag="st")
                    nc.vector.bn_stats(
                        out=st[:, 0, :],
                        in_=row.rearrange("p (a b) -> p a b", b=512)[:, 0, :])
                    nc.vector.bn_stats(
                        out=st[:, 1, :],
                        in_=row.rearrange("p (a b) -> p a b", b=512)[:, 1, :])
                    mv = p45.tile([P, 2], F32, tag="mv")
                    nc.vector.bn_aggr(out=mv, in_=st)
                    sd = p45.tile([P, 1], F32, tag="sd")
                    nc.scalar.activation(
                        out=sd, in_=mv[:, 1:2], func=Act.Sqrt,
                        scale=float(S) / float(S - 1))
                    sde = p45.tile([P, 1], F32, tag="sde")
                    nc.vector.tensor_scalar(
                        out=sde, in0=sd, scalar1=EPS, scalar2=None, op0=Alu.add)
                    r2 = p45.tile([P, 1], F32, tag="r2")
                    nc.vector.reciprocal(out=r2, in_=sde)
                    x2b = p45.tile([P, D], BF16, tag="x2b")
                    nc.vector.tensor_scalar(
                        out=x2b, in0=row, scalar1=mv[:, 0:1], scalar2=r2,
                        op0=Alu.subtract, op1=Alu.mult)
                    nc.sync.dma_start(
                        out=x2t[:, :, m * P:(m + 1) * P], in_=x2b, transpose=True)
            xr_cm.__exit__(None, None, None)
            wopre_cm.__exit__(None, None, None)
            attl_cm.__exit__(None, None, None)

            # ---------------- P6 + P7 interleaved by seq half ----------------
            with tc.tile_pool(name="ffn", bufs=1) as ffnp, \
                 tc.tile_pool(name="w1p", bufs=3) as w1p, \
                 tc.tile_pool(name="w2w", bufs=1) as w2w, \
                 tc.tile_pool(name="yst", bufs=3) as yst, \
                 tc.tile_pool(name="pf", bufs=4, space="PSUM") as pf:
                ht_lo = ffnp.tile([P, KD, S], BF16, tag="ht_lo")
                ht_hi = ffnp.tile([P, KD, S], BF16, tag="ht_hi")
                ht = [ht_lo, ht_hi]
                w2_sl = []
                for kf in range(KF):
                    t = w2w.tile([P, D], BF16, tag=f"w2_{kf}")
                    nc.sync.dma_start(out=t, in_=w2_d[kf])
                    w2_sl.append(t)
                for n in range(2):
                    for f in range(KF):
                        t8 = w1p.tile([P, KD, P], BF16, tag="w18")
                        nc.sync.dma_start(
                            out=t8,
                            in_=w1_d[:, :, f * P:(f + 1) * P].rearrange("k p m -> p k m"))
                        ps = pf.tile([P, 512], F32, tag="mm")
                        for k in range(KD):
                            nc.tensor.matmul(
                                ps, t8[:, k, :],
                                x2t[:, k, n * 512:(n + 1) * 512],
                                start=(k == 0), stop=(k == KD - 1))
                        nc.vector.tensor_scalar(
                            out=ht[f // KD][:, f % KD, n * 512:(n + 1) * 512],
                            in0=ps, scalar1=smalls[:, C_B1 + f:C_B1 + f + 1],
                            scalar2=0.0, op0=Alu.add, op1=Alu.max)
                    for m in range(4 * n, 4 * n + 4):
                        for nn in range(2):
                            ps = pf.tile([P, 512], F32, tag="mm")
                            for kf in range(KF):
                                nc.tensor.matmul(
                                    ps, ht[kf // KD][:, kf % KD, m * P:(m + 1) * P],
                                    w2_sl[kf][:, nn * 512:(nn + 1) * 512],
                                    start=(kf == 0), stop=(kf == KF - 1))
                            yt = yst.tile([P, 512], F32, tag="yt")
                            nc.vector.tensor_add(
                                out=yt, in0=ps,
                                in1=out1[:, m, nn * 512:(nn + 1) * 512])
                            if has_b2:
                                nc.vector.tensor_add(
                                    out=yt, in0=yt,
                                    in1=b2B[:, nn * 512:(nn + 1) * 512])
                            nc.sync.dma_start(
                                out=y_d[m, :, nn * 512:(nn + 1) * 512], in_=yt)

    nc.compile()
    return nc


def _col_tiles(v, ncols):
    """[N] -> [128, ncols] with element 128*j + i at [i, j]."""
    return np.ascontiguousarray(v.reshape(ncols, P).T)


def kernel(x, mask, n1_a, n1_b, n2_a, n2_b, wq, bq, wk, bk, wv, bv,
           wo, bo, w1, b1, w2, b2):
    global LAST_RESULT
    x = np.asarray(x, dtype=np.float32)
    mask = np.asarray(mask)
    f32 = lambda a: np.asarray(a, dtype=np.float32)
    n1_a, n1_b, n2_a, n2_b = map(f32, (n1_a, n1_b, n2_a, n2_b))
    wq, bq, wk, bk, wv, bv = map(f32, (wq, bq, wk, bk, wv, bv))
    wo, bo, w1, b1, w2, b2 = map(f32, (wo, bo, w1, b1, w2, b2))
    B = x.shape[0]
    assert x.shape == (B, S, D) and B == 8

    # fold LN affine params into following matmuls
    wq_e = n1_a[:, None] * wq
    wk_e = n1_a[:, None] * wk
    wv_e = n1_a[:, None] * wv
    bq_e = n1_b @ wq + bq
    bk_e = n1_b @ wk + bk
    bv_e = n1_b @ wv + bv
    w1_e = n2_a[:, None] * w1
    b1_e = n2_b @ w1 + b1

    # LN1 stats on host (input-only reduction)
    mu1 = x.mean(axis=-1, dtype=np.float32)                # [B, S]
    sd1 = x.std(axis=-1, ddof=1, dtype=np.float32)         # [B, S]
    r1 = 1.0 / (sd1 + EPS)
    maskb = np.where(mask[:, 0, :] == 0, np.float32(-1e5), np.float32(0.0))

    flags = (bool(bv_e.any()), bool(bo.any()), bool(b2.any()))
    if flags not in _CACHE:
        _CACHE[flags] = _build(flags)
    nc = _CACHE[flags]

    bf = lambda a: np.ascontiguousarray(a).astype(BF16NP)
    wq_t = bf(wq_e.reshape(KD, P, D))
    wk_t = bf(wk_e.reshape(KD, P, D))
    wv_t = bf(wv_e.reshape(KD, P, D))
    wo_t = bf(wo.reshape(KD, P, D))
    w1_t = bf(w1_e.reshape(KD, P, F))
    w2_t = bf(w2.reshape(KF, P, D))
    bq_c = _col_tiles(bq_e, KD)
    bk_c = _col_tiles(bk_e, KD)
    b1_c = _col_tiles(b1_e, KF)

    in_maps = []
    for b in range(B):
        smalls = np.zeros((P, 56), dtype=np.float32)
        smalls[:, C_MU:C_MU + NT] = _col_tiles(mu1[b], NT)
        smalls[:, C_R1:C_R1 + NT] = _col_tiles(r1[b], NT)
        smalls[:, C_MB:C_MB + NT] = _col_tiles(maskb[b], NT)
        smalls[:, C_BQ:C_BQ + KD] = bq_c
        smalls[:, C_BK:C_BK + KD] = bk_c
        smalls[:, C_B1:C_B1 + KF] = b1_c
        m = {
            "x": np.ascontiguousarray(x[b].reshape(NT, P, D)),
            "smalls": smalls,
            "wq": wq_t, "wk": wk_t, "wv": wv_t, "wo": wo_t,
            "w1": w1_t, "w2": w2_t,
        }
        if flags[0]:
            m["bv"] = bv_e.reshape(1, D).astype(np.float32)
        if flags[1]:
            m["bo"] = bo.reshape(1, D).astype(np.float32)
        if flags[2]:
            m["b2"] = b2.reshape(1, D).astype(np.float32)
        in_maps.append(m)

    res = run_bass_kernel_spmd(nc, in_maps, core_ids=list(range(8)))
    LAST_RESULT = res
    out = np.stack([res.results[b]["y"].reshape(S, D) for b in range(B)])
    return out


# revision 20
# speedup vs baseline: 1.1515x; 1.1515x over previous
"""Trainium2 Bass kernel for nn_EncoderLayer (B=8, S=1024, D=1024, H=16, FF=2048).

Sharding: data-parallel over batch — core i handles batch element i. No
collectives. GEMMs run in bf16 (1 cyc/row on PE); residual stream f32.

Per-core dataflow (S=seq, D=feat; P=128 partitions):
  P1  LN1 apply (host-computed mean/rstd) -> x2j bf16, DMA-transpose -> x2t
  V   vaug [S, H, 65] bf16 (65th column = ones; softmax denominator for free)
  P3  per head-pair pr: QT/KT = w^T @ x2t (SBUF-resident, bf16),
      per j: scoresT = K @ Q^T (row-packed 64x2 heads), exp via ACT with
      per-partition mask bias, attnT[65, S] += [V|1]^T @ expT.
      Denominators: PE-transpose the two [1,S] rows -> [128,16], one cheap
      DVE reciprocal, DMA round-trip to broadcast [64,S], normalize on evac.
  P4  out-proj O = catT^T @ wo (+ residual x) -> out1 seq-major (f32)
  P5  LN2 (bn_stats) -> x2b bf16, DMA-transpose -> x2t (reused)
  P6  HT = w1^T @ x2bt, relu (+b1) -> ht bf16
  P7  out2 = ht^T @ w2 (+ out1 residual) -> y
"""
import sys

sys.path.insert(0, "/opt/trn_rl_repo")

import numpy as np
import ml_dtypes

import concourse.bass as bass
import concourse.mybir as mybir
from concourse import bacc
from concourse.tile import TileContext
from concourse.bass_utils import run_bass_kernel_spmd
from concourse.masks import make_identity

P = 128
S = 1024
D = 1024
H = 16
DK = 64
F = 2048
NT = S // P   # seq tiles
KD = D // P   # feature k-tiles
KF = F // P   # ff k-tiles
EPS = 1e-6

F32 = mybir.dt.float32
BF16 = mybir.dt.bfloat16
Alu = mybir.AluOpType
Act = mybir.ActivationFunctionType
BF16NP = ml_dtypes.bfloat16

# smalls layout (columns of a [128, 56] tensor)
C_MU, C_R1, C_MB, C_BQ, C_BK, C_B1 = 0, 8, 16, 24, 32, 40  # b1 gets 16 cols

_CACHE = {}
LAST_RESULT = None


def _build(flags):
    has_bv, has_bo, has_b2 = flags
    nc = bacc.Bacc()

    x_d = nc.dram_tensor("x", [NT, P, D], F32, kind="ExternalInput")
    sm_d = nc.dram_tensor("smalls", [P, 56], F32, kind="ExternalInput")
    wq_d = nc.dram_tensor("wq", [KD, P, D], BF16, kind="ExternalInput")
    wk_d = nc.dram_tensor("wk", [KD, P, D], BF16, kind="ExternalInput")
    wv_d = nc.dram_tensor("wv", [KD, P, D], BF16, kind="ExternalInput")
    wo_d = nc.dram_tensor("wo", [KD, P, D], BF16, kind="ExternalInput")
    w1_d = nc.dram_tensor("w1", [KD, P, F], BF16, kind="ExternalInput")
    w2_d = nc.dram_tensor("w2", [KF, P, D], BF16, kind="ExternalInput")
    if has_bv:
        bv_d = nc.dram_tensor("bv", [1, D], F32, kind="ExternalInput")
    if has_bo:
        bo_d = nc.dram_tensor("bo", [1, D], F32, kind="ExternalInput")
    if has_b2:
        b2_d = nc.dram_tensor("b2", [1, D], F32, kind="ExternalInput")
    y_d = nc.dram_tensor("y", [NT, P, D], F32, kind="ExternalOutput")

    rd_d = nc.dram_tensor("rd_scratch", [H, S], F32)
    dn_d = nc.dram_tensor("dn_scratch", [H, S], F32)

    with TileContext(nc) as tc:
        with tc.tile_pool(name="const", bufs=1) as constp, \
             tc.tile_pool(name="big", bufs=1) as bigp:
            smalls = constp.tile([P, 56], F32)
            nc.sync.dma_start(out=smalls, in_=sm_d[:, :])
            ident = constp.tile([P, P], BF16)
            make_identity(nc, ident)

            def bias_bcast(dram_row):
                src_ap = dram_row[0:1, :]
                bc_ap = bass.AP(tensor=src_ap.tensor, offset=src_ap.offset,
                                ap=[[0, P]] + list(src_ap.ap)[1:])
                bc = constp.tile([P, D], F32)
                nc.sync.dma_start(out=bc, in_=bc_ap)
                return bc

            bvB = bias_bcast(bv_d) if has_bv else None
            boB = bias_bcast(bo_d) if has_bo else None
            b2B = bias_bcast(b2_d) if has_b2 else None

            out1 = bigp.tile([P, NT, D], F32, tag="out1")
            x2t = bigp.tile([P, KD, S], BF16, tag="x2t")

            attl_cm = tc.tile_pool(name="attl", bufs=1)
            attl = attl_cm.__enter__()
            vaug = attl.tile([P, NT, H, 65], BF16, tag="vaug")
            cat = attl.tile([P, KD, S], BF16, tag="cat")

            # ---------------- P1: LN1 apply + PE transpose ----------------
            with tc.tile_pool(name="p1", bufs=3) as p1, \
                 tc.tile_pool(name="pstr", bufs=2, space="PSUM") as pstr:
                for j in range(NT):
                    xj = p1.tile([P, D], F32, tag="xj")
                    nc.sync.dma_start(out=xj, in_=x_d[j])
                    x2j = p1.tile([P, D], BF16, tag="x2j")
                    nc.vector.tensor_scalar(
                        out=x2j, in0=xj,
                        scalar1=smalls[:, C_MU + j:C_MU + j + 1],
                        scalar2=smalls[:, C_R1 + j:C_R1 + j + 1],
                        op0=Alu.subtract, op1=Alu.mult)
                    for a in range(2):
                        ps = pstr.tile([P, 512], BF16, tag="tr")
                        for q in range(4):
                            i = 4 * a + q
                            nc.tensor.transpose(
                                ps[:, q * P:(q + 1) * P],
                                x2j[:, i * P:(i + 1) * P], ident)
                        nc.vector.tensor_copy(
                            out=x2t[:, 4 * a:4 * a + 4, j * P:(j + 1) * P],
                            in_=ps.rearrange("p (a b) -> p a b", b=P))

            # ---------------- V: vaug [S, H, 65] ----------------
            with tc.tile_pool(name="wvp", bufs=1) as wvp, \
                 tc.tile_pool(name="psv", bufs=4, space="PSUM") as psv:
                ones16 = constp.tile([P, H], F32)
                nc.vector.memset(ones16, 1.0)
                for i in range(NT):
                    nc.vector.tensor_copy(
                        out=vaug[:, i, :, 64:65],
                        in_=ones16.rearrange("p (h o) -> p h o", o=1))
                wv_sl = []
                for k in range(KD):
                    t = wvp.tile([P, D], BF16, tag=f"wv{k}")
                    nc.scalar.dma_start(out=t, in_=wv_d[k])
                    wv_sl.append(t)
                for n in range(2):
                    for i in range(NT):
                        ps = psv.tile([P, 512], F32, tag="mm")
                        for k in range(KD):
                            nc.tensor.matmul(
                                ps, x2t[:, k, i * P:(i + 1) * P],
                                wv_sl[k][:, n * 512:(n + 1) * 512],
                                start=(k == 0), stop=(k == KD - 1))
                        dst = vaug[:, i, 8 * n:8 * n + 8, 0:64]
                        if has_bv:
                            nc.vector.tensor_add(
                                out=dst, in0=ps.rearrange("p (h c) -> p h c", c=64),
                                in1=bvB[:, n * 512:(n + 1) * 512].rearrange(
                                    "p (h c) -> p h c", c=64))
                        else:
                            nc.vector.tensor_copy(
                                out=dst, in_=ps.rearrange("p (h c) -> p h c", c=64))

            # -------- P3: per head-pair QT/KT + attention --------
            wopre_cm = tc.tile_pool(name="wopre", bufs=1)
            wopre = wopre_cm.__enter__()
            xr_cm = tc.tile_pool(name="xr", bufs=1)
            xrp = xr_cm.__enter__()
            wo_tiles = {}
            xm_sl = []
            with tc.tile_pool(name="wp", bufs=2) as wp, \
                 tc.tile_pool(name="qk", bufs=2) as qkp, \
                 tc.tile_pool(name="ep", bufs=3) as ep, \
                 tc.tile_pool(name="dn", bufs=2) as dnp, \
                 tc.tile_pool(name="rr", bufs=2) as rrp, \
                 tc.tile_pool(name="rb", bufs=2) as rbp, \
                 tc.tile_pool(name="sc", bufs=2, space="PSUM") as scp, \
                 tc.tile_pool(name="at", bufs=2, space="PSUM") as atp:
                for pr in range(KD):
                    hA, hB = 2 * pr, 2 * pr + 1
                    # --- QT / KT for this head pair (SBUF-resident bf16) ---
                    wq8 = wp.tile([P, KD, P], BF16, tag="wq8")
                    nc.scalar.dma_start(
                        out=wq8,
                        in_=wq_d[:, :, pr * P:(pr + 1) * P].rearrange("k p m -> p k m"))
                    wk8 = wp.tile([P, KD, P], BF16, tag="wk8")
                    nc.scalar.dma_start(
                        out=wk8,
                        in_=wk_d[:, :, pr * P:(pr + 1) * P].rearrange("k p m -> p k m"))
                    qps = atp.tile([P, S], F32, tag="at")
                    for n in range(2):
                        for k in range(KD):
                            nc.tensor.matmul(
                                qps[:, n * 512:(n + 1) * 512], wq8[:, k, :],
                                x2t[:, k, n * 512:(n + 1) * 512],
                                start=(k == 0), stop=(k == KD - 1))
                    qtp = qkp.tile([P, S], BF16, tag="qt")
                    nc.vector.tensor_scalar(
                        out=qtp, in0=qps,
                        scalar1=smalls[:, C_BQ + pr:C_BQ + pr + 1], scalar2=None,
                        op0=Alu.add)
                    kps = atp.tile([P, S], F32, tag="at")
                    for n in range(2):
                        for k in range(KD):
                            nc.tensor.matmul(
                                kps[:, n * 512:(n + 1) * 512], wk8[:, k, :],
                                x2t[:, k, n * 512:(n + 1) * 512],
                                start=(k == 0), stop=(k == KD - 1))
                    ktp = qkp.tile([P, S], BF16, tag="kt")
                    nc.vector.tensor_scalar(
                        out=ktp, in0=kps,
                        scalar1=smalls[:, C_BK + pr:C_BK + pr + 1], scalar2=None,
                        op0=Alu.add)

                    # --- attention over key tiles j ---
                    aA = atp.tile([P, S], F32, tag="at")
                    aB = atp.tile([P, S], F32, tag="at")
                    for j in range(NT):
                        scA = scp.tile([P, S], F32, tag="sc")
                        scB = scp.tile([P, S], F32, tag="sc")
                        for n in range(2):
                            nc.tensor.matmul(
                                scA[:, n * 512:(n + 1) * 512],
                                ktp[0:64, j * P:(j + 1) * P],
                                qtp[0:64, n * 512:(n + 1) * 512],
                                start=True, stop=True, tile_position=(0, 0))
                            nc.tensor.matmul(
                                scB[:, n * 512:(n + 1) * 512],
                                ktp[64:P, j * P:(j + 1) * P],
                                qtp[64:P, n * 512:(n + 1) * 512],
                                start=True, stop=True, tile_position=(64, 0))
                        eA = ep.tile([P, S], BF16, tag="exp")
                        nc.scalar.activation(
                            out=eA, in_=scA, func=Act.Exp,
                            bias=smalls[:, C_MB + j:C_MB + j + 1], scale=0.125)
                        eB = ep.tile([P, S], BF16, tag="exp")
                        nc.scalar.activation(
                            out=eB, in_=scB, func=Act.Exp,
                            bias=smalls[:, C_MB + j:C_MB + j + 1], scale=0.125)
                        for n in range(2):
                            nc.tensor.matmul(
                                aA[0:65, n * 512:(n + 1) * 512],
                                vaug[:, j, hA, :],
                                eA[:, n * 512:(n + 1) * 512],
                                start=(j == 0), stop=(j == NT - 1))
                            nc.tensor.matmul(
                                aB[0:65, n * 512:(n + 1) * 512],
                                vaug[:, j, hB, :],
                                eB[:, n * 512:(n + 1) * 512],
                                start=(j == 0), stop=(j == NT - 1))

                    # evacuate attn psum immediately (frees the at-pool banks
                    # for the next head pair's QT/KT)
                    cpA = dnp.tile([65, S], F32, tag="cpA")
                    nc.vector.tensor_copy(out=cpA, in_=aA[0:65, :])
                    cpB = dnp.tile([65, S], F32, tag="cpB")
                    nc.vector.tensor_copy(out=cpB, in_=aB[0:65, :])

                    # --- denominators: row 64 -> DRAM -> strided gather
                    #     [p, a] -> one cheap recip -> DRAM -> bcast ---
                    nc.sync.dma_start(out=dn_d[hA:hA + 1, :], in_=cpA[64:65, :])
                    nc.sync.dma_start(out=dn_d[hB:hB + 1, :], in_=cpB[64:65, :])
                    rrs = rrp.tile([P, 2, NT], F32, tag="rrs")
                    for hb in (0, 1):
                        g_ap = bass.AP(tensor=dn_d[0:1, :].tensor,
                                       offset=(2 * pr + hb) * S,
                                       ap=[[1, P], [P, NT]])
                        nc.sync.dma_start(out=rrs[:, hb, :], in_=g_ap)
                    rrinv = rrp.tile([P, 2, NT], F32, tag="rr")
                    nc.vector.reciprocal(out=rrinv, in_=rrs)

                    def bcast_recip(hb):
                        h = 2 * pr + hb
                        dst = rd_d[h:h + 1, :].rearrange("o (a p) -> (o p) a", p=P)
                        nc.sync.dma_start(out=dst, in_=rrinv[:, hb, :])
                        s_ap = rd_d[h:h + 1, :]
                        bc_ap = bass.AP(tensor=s_ap.tensor, offset=s_ap.offset,
                                        ap=[[0, 64]] + list(s_ap.ap)[1:])
                        rbt = rbp.tile([64, S], F32, tag="rb")
                        nc.sync.dma_start(out=rbt, in_=bc_ap)
                        return rbt

                    rbA = bcast_recip(0)
                    nc.vector.tensor_mul(
                        out=cat[0:64, pr, :], in0=cpA[0:64, :], in1=rbA)
                    rbB = bcast_recip(1)
                    stg = dnp.tile([64, S], BF16, tag="stg")
                    nc.vector.tensor_mul(out=stg, in0=cpB[0:64, :], in1=rbB)
                    nc.sync.dma_start(out=cat[64:P, pr, :], in_=stg)

                    # prefetch wo and residual x while attention runs
                    if pr < 4:
                        for k in (2 * pr, 2 * pr + 1):
                            t = wopre.tile([P, D], BF16, tag=f"wo{k}", name=f"wo{k}")
                            nc.scalar.dma_start(out=t, in_=wo_d[k])
                            wo_tiles[k] = t
                    else:
                        for m in (2 * (pr - 4), 2 * (pr - 4) + 1):
                            xm = xrp.tile([P, D], F32, tag=f"xm{m}", name=f"xm{m}")
                            nc.sync.dma_start(out=xm, in_=x_d[m])
                            xm_sl.append(xm)

            # ---------------- P4 + P5 merged m-loop ----------------
            with tc.tile_pool(name="p45", bufs=3) as p45, \
                 tc.tile_pool(name="pstr2", bufs=2, space="PSUM") as pstr2, \
                 tc.tile_pool(name="pm", bufs=4, space="PSUM") as pm:
                for m in range(NT):
                    xm = xm_sl[m]
                    for n in range(2):
                        ps = pm.tile([P, 512], F32, tag="mm")
                        for k in range(KD):
                            nc.tensor.matmul(
                                ps, cat[:, k, m * P:(m + 1) * P],
                                wo_tiles[k][:, n * 512:(n + 1) * 512],
                                start=(k == 0), stop=(k == KD - 1))
                        dst = out1[:, m, n * 512:(n + 1) * 512]
                        nc.vector.tensor_add(
                            out=dst, in0=ps, in1=xm[:, n * 512:(n + 1) * 512])
                        if has_bo:
                            nc.vector.tensor_add(
                                out=dst, in0=dst, in1=boB[:, n * 512:(n + 1) * 512])
                    # LN2 for this m
                    row = out1[:, m, :]
                    st = p45.tile([P, 2, 6], F32, tag="st")
                    nc.vector.bn_stats(
                        out=st[:, 0, :],
                        in_=row.rearrange("p (a b) -> p a b", b=512)[:, 0, :])
                    nc.vector.bn_stats(
                        out=st[:, 1, :],
                        in_=row.rearrange("p (a b) -> p a b", b=512)[:, 1, :])
                    mv = p45.tile([P, 2], F32, tag="mv")
                    nc.vector.bn_aggr(out=mv, in_=st)
                    sd = p45.tile([P, 1], F32, tag="sd")
                    nc.scalar.activation(
                        out=sd, in_=mv[:, 1:2], func=Act.Sqrt,
                        scale=float(S) / float(S - 1))
                    sde = p45.tile([P, 1], F32, tag="sde")
                    nc.vector.tensor_scalar(
                        out=sde, in0=sd, scalar1=EPS, scalar2=None, op0=Alu.add)
                    r2 = p45.tile([P, 1], F32, tag="r2")
                    nc.vector.reciprocal(out=r2, in_=sde)
                    x2b = p45.tile([P, D], BF16, tag="x2b")
                    nc.vector.tensor_scalar(
                        out=x2b, in0=row, scalar1=mv[:, 0:1], scalar2=r2,
                        op0=Alu.subtract, op1=Alu.mult)
                    for a in range(2):
                        ps2 = pstr2.tile([P, 512], BF16, tag="tr")
                        for q in range(4):
                            i = 4 * a + q
                            nc.tensor.transpose(
                                ps2[:, q * P:(q + 1) * P],
                                x2b[:, i * P:(i + 1) * P], ident)
                        nc.vector.tensor_copy(
                            out=x2t[:, 4 * a:4 * a + 4, m * P:(m + 1) * P],
                            in_=ps2.rearrange("p (a b) -> p a b", b=P))
            xr_cm.__exit__(None, None, None)
            wopre_cm.__exit__(None, None, None)
            attl_cm.__exit__(None, None, None)

            # ---------------- P6 + P7 interleaved by seq half ----------------
            with tc.tile_pool(name="ffn", bufs=1) as ffnp, \
                 tc.tile_pool(name="w1p", bufs=3) as w1p, \
                 tc.tile_pool(name="w2w", bufs=1) as w2w, \
                 tc.tile_pool(name="yst", bufs=3) as yst, \
                 tc.tile_pool(name="pf", bufs=4, space="PSUM") as pf:
                ht_lo = ffnp.tile([P, KD, S], BF16, tag="ht_lo")
                ht_hi = ffnp.tile([P, KD, S], BF16, tag="ht_hi")
                ht = [ht_lo, ht_hi]
                w2_sl = []
                for kf in range(KF):
                    t = w2w.tile([P, D], BF16, tag=f"w2_{kf}")
                    nc.sync.dma_start(out=t, in_=w2_d[kf])
                    w2_sl.append(t)
                for n in range(2):
                    for f in range(KF):
                        t8 = w1p.tile([P, KD, P], BF16, tag="w18")
                        nc.scalar.dma_start(
                            out=t8,
                            in_=w1_d[:, :, f * P:(f + 1) * P].rearrange("k p m -> p k m"))
                        ps = pf.tile([P, 512], F32, tag="mm")
                        for k in range(KD):
                            nc.tensor.matmul(
                                ps, t8[:, k, :],
                                x2t[:, k, n * 512:(n + 1) * 512],
                                start=(k == 0), stop=(k == KD - 1))
                        nc.vector.tensor_scalar(
                            out=ht[f // KD][:, f % KD, n * 512:(n + 1) * 512],
                            in0=ps, scalar1=smalls[:, C_B1 + f:C_B1 + f + 1],
                            scalar2=0.0, op0=Alu.add, op1=Alu.max)
                    for m in range(4 * n, 4 * n + 4):
                        for nn in range(2):
                            ps = pf.tile([P, 512], F32, tag="mm")
                            for kf in range(KF):
                                nc.tensor.matmul(
                                    ps, ht[kf // KD][:, kf % KD, m * P:(m + 1) * P],
                                    w2_sl[kf][:, nn * 512:(nn + 1) * 512],
                                    start=(kf == 0), stop=(kf == KF - 1))
                            yt = yst.tile([P, 512], F32, tag="yt")
                            nc.vector.tensor_add(
                                out=yt, in0=ps,
                                in1=out1[:, m, nn * 512:(nn + 1) * 512])
                            if has_b2:
                                nc.vector.tensor_add(
                                    out=yt, in0=yt,
                                    in1=b2B[:, nn * 512:(nn + 1) * 512])
                            nc.sync.dma_start(
                                out=y_d[m, :, nn * 512:(nn + 1) * 512], in_=yt)

    nc.compile()
    return nc


def _col_tiles(v, ncols):
    """[N] -> [128, ncols] with element 128*j + i at [i, j]."""
    return np.ascontiguousarray(v.reshape(ncols, P).T)


def kernel(x, mask, n1_a, n1_b, n2_a, n2_b, wq, bq, wk, bk, wv, bv,
           wo, bo, w1, b1, w2, b2):
    global LAST_RESULT
    x = np.asarray(x, dtype=np.float32)
    mask = np.asarray(mask)
    f32 = lambda a: np.asarray(a, dtype=np.float32)
    n1_a, n1_b, n2_a, n2_b = map(f32, (n1_a, n1_b, n2_a, n2_b))
    wq, bq, wk, bk, wv, bv = map(f32, (wq, bq, wk, bk, wv, bv))
    wo, bo, w1, b1, w2, b2 = map(f32, (wo, bo, w1, b1, w2, b2))
    B = x.shape[0]
    assert x.shape == (B, S, D) and B == 8

    # fold LN affine params into following matmuls
    wq_e = n1_a[:, None] * wq
    wk_e = n1_a[:, None] * wk
    wv_e = n1_a[:, None] * wv
    bq_e = n1_b @ wq + bq
    bk_e = n1_b @ wk + bk
    bv_e = n1_b @ wv + bv
    w1_e = n2_a[:, None] * w1
    b1_e = n2_b @ w1 + b1

    # LN1 stats on host (input-only reduction)
    mu1 = x.mean(axis=-1, dtype=np.float32)                # [B, S]
    sd1 = x.std(axis=-1, ddof=1, dtype=np.float32)         # [B, S]
    r1 = 1.0 / (sd1 + EPS)
    maskb = np.where(mask[:, 0, :] == 0, np.float32(-1e5), np.float32(0.0))

    flags = (bool(bv_e.any()), bool(bo.any()), bool(b2.any()))
    if flags not in _CACHE:
        _CACHE[flags] = _build(flags)
    nc = _CACHE[flags]

    bf = lambda a: np.ascontiguousarray(a).astype(BF16NP)
    wq_t = bf(wq_e.reshape(KD, P, D))
    wk_t = bf(wk_e.reshape(KD, P, D))
    wv_t = bf(wv_e.reshape(KD, P, D))
    wo_t = bf(wo.reshape(KD, P, D))
    w1_t = bf(w1_e.reshape(KD, P, F))
    w2_t = bf(w2.reshape(KF, P, D))
    bq_c = _col_tiles(bq_e, KD)
    bk_c = _col_tiles(bk_e, KD)
    b1_c = _col_tiles(b1_e, KF)

    in_maps = []
    for b in range(B):
        smalls = np.zeros((P, 56), dtype=np.float32)
        smalls[:, C_MU:C_MU + NT] = _col_tiles(mu1[b], NT)
        smalls[:, C_R1:C_R1 + NT] = _col_tiles(r1[b], NT)
        smalls[:, C_MB:C_MB + NT] = _col_tiles(maskb[b], NT)
        smalls[:, C_BQ:C_BQ + KD] = bq_c
        smalls[:, C_BK:C_BK + KD] = bk_c
        smalls[:, C_B1:C_B1 + KF] = b1_c
        m = {
            "x": np.ascontiguousarray(x[b].reshape(NT, P, D)),
            "smalls": smalls,
            "wq": wq_t, "wk": wk_t, "wv": wv_t, "wo": wo_t,
            "w1": w1_t, "w2": w2_t,
        }
        if flags[0]:
            m["bv"] = bv_e.reshape(1, D).astype(np.float32)
        if flags[1]:
            m["bo"] = bo.reshape(1, D).astype(np.float32)
        if flags[2]:
            m["b2"] = b2.reshape(1, D).astype(np.float32)
        in_maps.append(m)

    res = run_bass_kernel_spmd(nc, in_maps, core_ids=list(range(8)))
    LAST_RESULT = res
    out = np.stack([res.results[b]["y"].reshape(S, D) for b in range(B)])
    return out


# revision 22
# speedup vs baseline: 1.3034x; 1.1319x over previous
"""Trainium2 Bass kernel for nn_EncoderLayer (B=8, S=1024, D=1024, H=16, FF=2048).

Sharding: data-parallel over batch — core i handles batch element i. No
collectives. GEMMs run in bf16 (1 cyc/row on PE); residual stream f32.

Per-core dataflow (S=seq, D=feat; P=128 partitions):
  P1  LN1 apply (host-computed mean/rstd) -> x2j bf16, DMA-transpose -> x2t
  V   vaug [S, H, 65] bf16 (65th column = ones; softmax denominator for free)
  P3  per head-pair pr: QT/KT = w^T @ x2t (SBUF-resident, bf16),
      per j: scoresT = K @ Q^T (row-packed 64x2 heads), exp via ACT with
      per-partition mask bias, attnT[65, S] += [V|1]^T @ expT.
      Denominators: PE-transpose the two [1,S] rows -> [128,16], one cheap
      DVE reciprocal, DMA round-trip to broadcast [64,S], normalize on evac.
  P4  out-proj O = catT^T @ wo (+ residual x) -> out1 seq-major (f32)
  P5  LN2 (bn_stats) -> x2b bf16, DMA-transpose -> x2t (reused)
  P6  HT = w1^T @ x2bt, relu (+b1) -> ht bf16
  P7  out2 = ht^T @ w2 (+ out1 residual) -> y
"""
import sys

sys.path.insert(0, "/opt/trn_rl_repo")

import numpy as np
import ml_dtypes

import concourse.bass as bass
import concourse.mybir as mybir
from concourse import bacc
from concourse.tile import TileContext
from concourse.bass_utils import run_bass_kernel_spmd
from concourse.masks import make_identity

P = 128
S = 1024
D = 1024
H = 16
DK = 64
F = 2048
NT = S // P   # seq tiles
KD = D // P   # feature k-tiles
KF = F // P   # ff k-tiles
EPS = 1e-6

F32 = mybir.dt.float32
BF16 = mybir.dt.bfloat16
Alu = mybir.AluOpType
Act = mybir.ActivationFunctionType
BF16NP = ml_dtypes.bfloat16

# smalls layout (columns of a [128, 56] tensor)
C_MU, C_R1, C_MB, C_BQ, C_BK, C_B1 = 0, 8, 16, 24, 32, 40  # b1 gets 16 cols

_CACHE = {}
LAST_RESULT = None


def _build(flags):
    has_bv, has_bo, has_b2 = flags
    nc = bacc.Bacc()

    x_d = nc.dram_tensor("x", [NT, P, D], F32, kind="ExternalInput")
    sm_d = nc.dram_tensor("smalls", [P, 56], F32, kind="ExternalInput")
    wq_d = nc.dram_tensor("wq", [KD, P, D], BF16, kind="ExternalInput")
    wk_d = nc.dram_tensor("wk", [KD, P, D], BF16, kind="ExternalInput")
    wv_d = nc.dram_tensor("wv", [KD, P, D], BF16, kind="ExternalInput")
    wo_d = nc.dram_tensor("wo", [KD, P, D], BF16, kind="ExternalInput")
    w1_d = nc.dram_tensor("w1", [KD, P, F], BF16, kind="ExternalInput")
    w2_d = nc.dram_tensor("w2", [KF, P, D], BF16, kind="ExternalInput")
    if has_bv:
        bv_d = nc.dram_tensor("bv", [1, D], F32, kind="ExternalInput")
    if has_bo:
        bo_d = nc.dram_tensor("bo", [1, D], F32, kind="ExternalInput")
    if has_b2:
        b2_d = nc.dram_tensor("b2", [1, D], F32, kind="ExternalInput")
    y_d = nc.dram_tensor("y", [NT, P, D], F32, kind="ExternalOutput")

    rd_d = nc.dram_tensor("rd_scratch", [H, S], F32)
    dn_d = nc.dram_tensor("dn_scratch", [H, S], F32)

    with TileContext(nc) as tc:
        with tc.tile_pool(name="const", bufs=1) as constp, \
             tc.tile_pool(name="big", bufs=1) as bigp:
            smalls = constp.tile([P, 56], F32)
            nc.sync.dma_start(out=smalls, in_=sm_d[:, :])
            ident = constp.tile([P, P], BF16)
            make_identity(nc, ident)

            def bias_bcast(dram_row):
                src_ap = dram_row[0:1, :]
                bc_ap = bass.AP(tensor=src_ap.tensor, offset=src_ap.offset,
                                ap=[[0, P]] + list(src_ap.ap)[1:])
                bc = constp.tile([P, D], F32)
                nc.sync.dma_start(out=bc, in_=bc_ap)
                return bc

            bvB = bias_bcast(bv_d) if has_bv else None
            boB = bias_bcast(bo_d) if has_bo else None
            b2B = bias_bcast(b2_d) if has_b2 else None

            out1 = bigp.tile([P, NT, D], F32, tag="out1")
            x2t = bigp.tile([P, KD, S], BF16, tag="x2t")

            attl_cm = tc.tile_pool(name="attl", bufs=1)
            attl = attl_cm.__enter__()
            vaug = attl.tile([P, NT, H, 65], BF16, tag="vaug")
            cat = attl.tile([P, KD, S], BF16, tag="cat")

            # ---------------- P1: LN1 apply + PE transpose ----------------
            with tc.tile_pool(name="p1", bufs=3) as p1, \
                 tc.tile_pool(name="pstr", bufs=2, space="PSUM") as pstr:
                for j in range(NT):
                    xj = p1.tile([P, D], F32, tag="xj")
                    nc.sync.dma_start(out=xj, in_=x_d[j])
                    x2j = p1.tile([P, D], BF16, tag="x2j")
                    nc.vector.tensor_scalar(
                        out=x2j, in0=xj,
                        scalar1=smalls[:, C_MU + j:C_MU + j + 1],
                        scalar2=smalls[:, C_R1 + j:C_R1 + j + 1],
                        op0=Alu.subtract, op1=Alu.mult)
                    for a in range(2):
                        ps = pstr.tile([P, 512], BF16, tag="tr")
                        for q in range(4):
                            i = 4 * a + q
                            nc.tensor.transpose(
                                ps[:, q * P:(q + 1) * P],
                                x2j[:, i * P:(i + 1) * P], ident)
                        nc.vector.tensor_copy(
                            out=x2t[:, 4 * a:4 * a + 4, j * P:(j + 1) * P],
                            in_=ps.rearrange("p (a b) -> p a b", b=P))

            # ---------------- V: vaug [S, H, 65] ----------------
            with tc.tile_pool(name="wvp", bufs=1) as wvp, \
                 tc.tile_pool(name="psv", bufs=4, space="PSUM") as psv:
                ones16 = constp.tile([P, H], F32)
                nc.vector.memset(ones16, 1.0)
                for i in range(NT):
                    nc.vector.tensor_copy(
                        out=vaug[:, i, :, 64:65],
                        in_=ones16.rearrange("p (h o) -> p h o", o=1))
                wv_sl = []
                for k in range(KD):
                    t = wvp.tile([P, D], BF16, tag=f"wv{k}")
                    nc.scalar.dma_start(out=t, in_=wv_d[k])
                    wv_sl.append(t)
                for n in range(2):
                    for i in range(NT):
                        ps = psv.tile([P, 512], F32, tag="mm")
                        for k in range(KD):
                            nc.tensor.matmul(
                                ps, x2t[:, k, i * P:(i + 1) * P],
                                wv_sl[k][:, n * 512:(n + 1) * 512],
                                start=(k == 0), stop=(k == KD - 1))
                        dst = vaug[:, i, 8 * n:8 * n + 8, 0:64]
                        if has_bv:
                            nc.vector.tensor_add(
                                out=dst, in0=ps.rearrange("p (h c) -> p h c", c=64),
                                in1=bvB[:, n * 512:(n + 1) * 512].rearrange(
                                    "p (h c) -> p h c", c=64))
                        else:
                            nc.vector.tensor_copy(
                                out=dst, in_=ps.rearrange("p (h c) -> p h c", c=64))

            # -------- P3: per head-pair QT/KT + attention --------
            wopre_cm = tc.tile_pool(name="wopre", bufs=1)
            wopre = wopre_cm.__enter__()
            xr_cm = tc.tile_pool(name="xr", bufs=1)
            xrp = xr_cm.__enter__()
            wo_tiles = {}
            xm_sl = []
            with tc.tile_pool(name="wp", bufs=2) as wp, \
                 tc.tile_pool(name="qk", bufs=2) as qkp, \
                 tc.tile_pool(name="ep", bufs=3) as ep, \
                 tc.tile_pool(name="dn", bufs=2) as dnp, \
                 tc.tile_pool(name="rr", bufs=2) as rrp, \
                 tc.tile_pool(name="rb", bufs=2) as rbp, \
                 tc.tile_pool(name="sc", bufs=2, space="PSUM") as scp, \
                 tc.tile_pool(name="at", bufs=2, space="PSUM") as atp:
                for pr in range(KD):
                    hA, hB = 2 * pr, 2 * pr + 1
                    # --- QT / KT for this head pair (SBUF-resident bf16) ---
                    wq8 = wp.tile([P, KD, P], BF16, tag="wq8")
                    nc.scalar.dma_start(
                        out=wq8,
                        in_=wq_d[:, :, pr * P:(pr + 1) * P].rearrange("k p m -> p k m"))
                    wk8 = wp.tile([P, KD, P], BF16, tag="wk8")
                    nc.scalar.dma_start(
                        out=wk8,
                        in_=wk_d[:, :, pr * P:(pr + 1) * P].rearrange("k p m -> p k m"))
                    qps = atp.tile([P, S], F32, tag="at")
                    for n in range(2):
                        for k in range(KD):
                            nc.tensor.matmul(
                                qps[:, n * 512:(n + 1) * 512], wq8[:, k, :],
                                x2t[:, k, n * 512:(n + 1) * 512],
                                start=(k == 0), stop=(k == KD - 1))
                    qtp = qkp.tile([P, S], BF16, tag="qt")
                    nc.vector.tensor_scalar(
                        out=qtp, in0=qps,
                        scalar1=smalls[:, C_BQ + pr:C_BQ + pr + 1], scalar2=None,
                        op0=Alu.add)
                    kps = atp.tile([P, S], F32, tag="at")
                    for n in range(2):
                        for k in range(KD):
                            nc.tensor.matmul(
                                kps[:, n * 512:(n + 1) * 512], wk8[:, k, :],
                                x2t[:, k, n * 512:(n + 1) * 512],
                                start=(k == 0), stop=(k == KD - 1))
                    ktp = qkp.tile([P, S], BF16, tag="kt")
                    nc.vector.tensor_scalar(
                        out=ktp, in0=kps,
                        scalar1=smalls[:, C_BK + pr:C_BK + pr + 1], scalar2=None,
                        op0=Alu.add)

                    # --- attention over key tiles j (software-pipelined:
                    #     scores/exp for j+1 are emitted before attn for j so
                    #     the in-order PE never waits on the scalar engine) ---
                    aA = atp.tile([P, S], F32, tag="at")
                    aB = atp.tile([P, S], F32, tag="at")
                    e_tiles = {}

                    def scores_exp(j):
                        scA = scp.tile([P, S], F32, tag="sc", name="scA")
                        scB = scp.tile([P, S], F32, tag="sc", name="scB")
                        for n in range(2):
                            nc.tensor.matmul(
                                scA[:, n * 512:(n + 1) * 512],
                                ktp[0:64, j * P:(j + 1) * P],
                                qtp[0:64, n * 512:(n + 1) * 512],
                                start=True, stop=True, tile_position=(0, 0))
                            nc.tensor.matmul(
                                scB[:, n * 512:(n + 1) * 512],
                                ktp[64:P, j * P:(j + 1) * P],
                                qtp[64:P, n * 512:(n + 1) * 512],
                                start=True, stop=True, tile_position=(64, 0))
                        eA = ep.tile([P, S], BF16, tag="exp", bufs=4, name="eA")
                        nc.scalar.activation(
                            out=eA, in_=scA, func=Act.Exp,
                            bias=smalls[:, C_MB + j:C_MB + j + 1], scale=0.125)
                        eB = ep.tile([P, S], BF16, tag="exp", bufs=4, name="eB")
                        nc.scalar.activation(
                            out=eB, in_=scB, func=Act.Exp,
                            bias=smalls[:, C_MB + j:C_MB + j + 1], scale=0.125)
                        e_tiles[j] = (eA, eB)

                    def attn(j):
                        eA, eB = e_tiles.pop(j)
                        for n in range(2):
                            nc.tensor.matmul(
                                aA[0:65, n * 512:(n + 1) * 512],
                                vaug[:, j, hA, :],
                                eA[:, n * 512:(n + 1) * 512],
                                start=(j == 0), stop=(j == NT - 1))
                            nc.tensor.matmul(
                                aB[0:65, n * 512:(n + 1) * 512],
                                vaug[:, j, hB, :],
                                eB[:, n * 512:(n + 1) * 512],
                                start=(j == 0), stop=(j == NT - 1))

                    scores_exp(0)
                    for j in range(1, NT):
                        scores_exp(j)
                        attn(j - 1)
                    attn(NT - 1)

                    # evacuate attn psum immediately (frees the at-pool banks
                    # for the next head pair's QT/KT)
                    cpA = dnp.tile([65, S], F32, tag="cpA")
                    nc.vector.tensor_copy(out=cpA, in_=aA[0:65, :])
                    cpB = dnp.tile([65, S], F32, tag="cpB")
                    nc.vector.tensor_copy(out=cpB, in_=aB[0:65, :])

                    # --- denominators: row 64 -> DRAM -> strided gather
                    #     [p, a] -> one cheap recip -> DRAM -> bcast ---
                    nc.sync.dma_start(out=dn_d[hA:hA + 1, :], in_=cpA[64:65, :])
                    nc.sync.dma_start(out=dn_d[hB:hB + 1, :], in_=cpB[64:65, :])
                    rrs = rrp.tile([P, 2, NT], F32, tag="rrs")
                    for hb in (0, 1):
                        g_ap = bass.AP(tensor=dn_d[0:1, :].tensor,
                                       offset=(2 * pr + hb) * S,
                                       ap=[[1, P], [P, NT]])
                        nc.sync.dma_start(out=rrs[:, hb, :], in_=g_ap)
                    rrinv = rrp.tile([P, 2, NT], F32, tag="rr")
                    nc.vector.reciprocal(out=rrinv, in_=rrs)

                    def bcast_recip(hb):
                        h = 2 * pr + hb
                        dst = rd_d[h:h + 1, :].rearrange("o (a p) -> (o p) a", p=P)
                        nc.sync.dma_start(out=dst, in_=rrinv[:, hb, :])
                        s_ap = rd_d[h:h + 1, :]
                        bc_ap = bass.AP(tensor=s_ap.tensor, offset=s_ap.offset,
                                        ap=[[0, 64]] + list(s_ap.ap)[1:])
                        rbt = rbp.tile([64, S], F32, tag="rb")
                        nc.sync.dma_start(out=rbt, in_=bc_ap)
                        return rbt

                    rbA = bcast_recip(0)
                    nc.vector.tensor_mul(
                        out=cat[0:64, pr, :], in0=cpA[0:64, :], in1=rbA)
                    rbB = bcast_recip(1)
                    stg = dnp.tile([64, S], BF16, tag="stg")
                    nc.vector.tensor_mul(out=stg, in0=cpB[0:64, :], in1=rbB)
                    nc.sync.dma_start(out=cat[64:P, pr, :], in_=stg)

                    # prefetch wo and residual x while attention runs
                    if pr < 4:
                        for k in (2 * pr, 2 * pr + 1):
                            t = wopre.tile([P, D], BF16, tag=f"wo{k}", name=f"wo{k}")
                            nc.scalar.dma_start(out=t, in_=wo_d[k])
                            wo_tiles[k] = t
                    else:
                        for m in (2 * (pr - 4), 2 * (pr - 4) + 1):
                            xm = xrp.tile([P, D], F32, tag=f"xm{m}", name=f"xm{m}")
                            nc.sync.dma_start(out=xm, in_=x_d[m])
                            xm_sl.append(xm)

            # ---------------- P4 + P5 merged m-loop ----------------
            with tc.tile_pool(name="p45", bufs=3) as p45, \
                 tc.tile_pool(name="pstr2", bufs=2, space="PSUM") as pstr2, \
                 tc.tile_pool(name="pm", bufs=4, space="PSUM") as pm:
                for m in range(NT):
                    xm = xm_sl[m]
                    for n in range(2):
                        ps = pm.tile([P, 512], F32, tag="mm")
                        # rotate k-order so the last matmul of tile m only
                        # needs cat[:, m] — absorbs the attention tail latency
                        for ki in range(KD):
                            k = (m + 1 + ki) % KD
                            nc.tensor.matmul(
                                ps, cat[:, k, m * P:(m + 1) * P],
                                wo_tiles[k][:, n * 512:(n + 1) * 512],
                                start=(ki == 0), stop=(ki == KD - 1))
                        dst = out1[:, m, n * 512:(n + 1) * 512]
                        nc.vector.tensor_add(
                            out=dst, in0=ps, in1=xm[:, n * 512:(n + 1) * 512])
                        if has_bo:
                            nc.vector.tensor_add(
                                out=dst, in0=dst, in1=boB[:, n * 512:(n + 1) * 512])
                    # LN2 for this m
                    row = out1[:, m, :]
                    st = p45.tile([P, 2, 6], F32, tag="st")
                    nc.vector.bn_stats(
                        out=st[:, 0, :],
                        in_=row.rearrange("p (a b) -> p a b", b=512)[:, 0, :])
                    nc.vector.bn_stats(
                        out=st[:, 1, :],
                        in_=row.rearrange("p (a b) -> p a b", b=512)[:, 1, :])
                    mv = p45.tile([P, 2], F32, tag="mv")
                    nc.vector.bn_aggr(out=mv, in_=st)
                    sd = p45.tile([P, 1], F32, tag="sd")
                    nc.scalar.activation(
                        out=sd, in_=mv[:, 1:2], func=Act.Sqrt,
                        scale=float(S) / float(S - 1))
                    sde = p45.tile([P, 1], F32, tag="sde")
                    nc.vector.tensor_scalar(
                        out=sde, in0=sd, scalar1=EPS, scalar2=None, op0=Alu.add)
                    r2 = p45.tile([P, 1], F32, tag="r2")
                    nc.vector.reciprocal(out=r2, in_=sde)
                    x2b = p45.tile([P, D], BF16, tag="x2b")
                    nc.vector.tensor_scalar(
                        out=x2b, in0=row, scalar1=mv[:, 0:1], scalar2=r2,
                        op0=Alu.subtract, op1=Alu.mult)
                    for a in range(2):
                        ps2 = pstr2.tile([P, 512], BF16, tag="tr")
                        for q in range(4):
                            i = 4 * a + q
                            nc.tensor.transpose(
                                ps2[:, q * P:(q + 1) * P],
                                x2b[:, i * P:(i + 1) * P], ident)
                        nc.vector.tensor_copy(
                            out=x2t[:, 4 * a:4 * a + 4, m * P:(m + 1) * P],
                            in_=ps2.rearrange("p (a b) -> p a b", b=P))
            xr_cm.__exit__(None, None, None)
            wopre_cm.__exit__(None, None, None)
            attl_cm.__exit__(None, None, None)

            # ---------------- P6 + P7 interleaved by seq half ----------------
            with tc.tile_pool(name="ffn", bufs=1) as ffnp, \
                 tc.tile_pool(name="w1p", bufs=3) as w1p, \
                 tc.tile_pool(name="w2w", bufs=1) as w2w, \
                 tc.tile_pool(name="yst", bufs=3) as yst, \
                 tc.tile_pool(name="pf", bufs=4, space="PSUM") as pf:
                ht_lo = ffnp.tile([P, KD, S], BF16, tag="ht_lo")
                ht_hi = ffnp.tile([P, KD, S], BF16, tag="ht_hi")
                ht = [ht_lo, ht_hi]
                w2_sl = []
                for kf in range(KF):
                    t = w2w.tile([P, D], BF16, tag=f"w2_{kf}")
                    nc.sync.dma_start(out=t, in_=w2_d[kf])
                    w2_sl.append(t)
                for n in range(2):
                    for f in range(KF):
                        t8 = w1p.tile([P, KD, P], BF16, tag="w18")
                        nc.scalar.dma_start(
                            out=t8,
                            in_=w1_d[:, :, f * P:(f + 1) * P].rearrange("k p m -> p k m"))
                        ps = pf.tile([P, 512], F32, tag="mm")
                        for k in range(KD):
                            nc.tensor.matmul(
                                ps, t8[:, k, :],
                                x2t[:, k, n * 512:(n + 1) * 512],
                                start=(k == 0), stop=(k == KD - 1))
                        nc.vector.tensor_scalar(
                            out=ht[f // KD][:, f % KD, n * 512:(n + 1) * 512],
                            in0=ps, scalar1=smalls[:, C_B1 + f:C_B1 + f + 1],
                            scalar2=0.0, op0=Alu.add, op1=Alu.max)
                    for m in range(4 * n, 4 * n + 4):
                        for nn in range(2):
                            ps = pf.tile([P, 512], F32, tag="mm")
                            for kf in range(KF):
                                nc.tensor.matmul(
                                    ps, ht[kf // KD][:, kf % KD, m * P:(m + 1) * P],
                                    w2_sl[kf][:, nn * 512:(nn + 1) * 512],
                                    start=(kf == 0), stop=(kf == KF - 1))
                            yt = yst.tile([P, 512], F32, tag="yt")
                            nc.vector.tensor_add(
                                out=yt, in0=ps,
                                in1=out1[:, m, nn * 512:(nn + 1) * 512])
                            if has_b2:
                                nc.vector.tensor_add(
                                    out=yt, in0=yt,
                                    in1=b2B[:, nn * 512:(nn + 1) * 512])
                            nc.sync.dma_start(
                                out=y_d[m, :, nn * 512:(nn + 1) * 512], in_=yt)

    nc.compile()
    return nc


def _col_tiles(v, ncols):
    """[N] -> [128, ncols] with element 128*j + i at [i, j]."""
    return np.ascontiguousarray(v.reshape(ncols, P).T)


def kernel(x, mask, n1_a, n1_b, n2_a, n2_b, wq, bq, wk, bk, wv, bv,
           wo, bo, w1, b1, w2, b2):
    global LAST_RESULT
    x = np.asarray(x, dtype=np.float32)
    mask = np.asarray(mask)
    f32 = lambda a: np.asarray(a, dtype=np.float32)
    n1_a, n1_b, n2_a, n2_b = map(f32, (n1_a, n1_b, n2_a, n2_b))
    wq, bq, wk, bk, wv, bv = map(f32, (wq, bq, wk, bk, wv, bv))
    wo, bo, w1, b1, w2, b2 = map(f32, (wo, bo, w1, b1, w2, b2))
    B = x.shape[0]
    assert x.shape == (B, S, D) and B == 8

    # fold LN affine params into following matmuls
    wq_e = n1_a[:, None] * wq
    wk_e = n1_a[:, None] * wk
    wv_e = n1_a[:, None] * wv
    bq_e = n1_b @ wq + bq
    bk_e = n1_b @ wk + bk
    bv_e = n1_b @ wv + bv
    w1_e = n2_a[:, None] * w1
    b1_e = n2_b @ w1 + b1

    # LN1 stats on host (input-only reduction)
    mu1 = x.mean(axis=-1, dtype=np.float32)                # [B, S]
    sd1 = x.std(axis=-1, ddof=1, dtype=np.float32)         # [B, S]
    r1 = 1.0 / (sd1 + EPS)
    maskb = np.where(mask[:, 0, :] == 0, np.float32(-1e5), np.float32(0.0))

    flags = (bool(bv_e.any()), bool(bo.any()), bool(b2.any()))
    if flags not in _CACHE:
        _CACHE[flags] = _build(flags)
    nc = _CACHE[flags]

    bf = lambda a: np.ascontiguousarray(a).astype(BF16NP)
    wq_t = bf(wq_e.reshape(KD, P, D))
    wk_t = bf(wk_e.reshape(KD, P, D))
    wv_t = bf(wv_e.reshape(KD, P, D))
    wo_t = bf(wo.reshape(KD, P, D))
    w1_t = bf(w1_e.reshape(KD, P, F))
    w2_t = bf(w2.reshape(KF, P, D))
    bq_c = _col_tiles(bq_e, KD)
    bk_c = _col_tiles(bk_e, KD)
    b1_c = _col_tiles(b1_e, KF)

    in_maps = []
    for b in range(B):
        smalls = np.zeros((P, 56), dtype=np.float32)
        smalls[:, C_MU:C_MU + NT] = _col_tiles(mu1[b], NT)
        smalls[:, C_R1:C_R1 + NT] = _col_tiles(r1[b], NT)
        smalls[:, C_MB:C_MB + NT] = _col_tiles(maskb[b], NT)
        smalls[:, C_BQ:C_BQ + KD] = bq_c
        smalls[:, C_BK:C_BK + KD] = bk_c
        smalls[:, C_B1:C_B1 + KF] = b1_c
        m = {
            "x": np.ascontiguousarray(x[b].reshape(NT, P, D)),
            "smalls": smalls,
            "wq": wq_t, "wk": wk_t, "wv": wv_t, "wo": wo_t,
            "w1": w1_t, "w2": w2_t,
        }
        if flags[0]:
            m["bv"] = bv_e.reshape(1, D).astype(np.float32)
        if flags[1]:
            m["bo"] = bo.reshape(1, D).astype(np.float32)
        if flags[2]:
            m["b2"] = b2.reshape(1, D).astype(np.float32)
        in_maps.append(m)

    res = run_bass_kernel_spmd(nc, in_maps, core_ids=list(range(8)))
    LAST_RESULT = res
    out = np.stack([res.results[b]["y"].reshape(S, D) for b in range(B)])
    return out


# revision 23
# speedup vs baseline: 1.4967x; 1.1483x over previous
"""Trainium2 Bass kernel for nn_EncoderLayer (B=8, S=1024, D=1024, H=16, FF=2048).

Sharding: data-parallel over batch — core i handles batch element i. No
collectives. GEMMs run in bf16 (1 cyc/row on PE); residual stream f32.

Per-core dataflow (S=seq, D=feat; P=128 partitions):
  P1  LN1 apply (host-computed mean/rstd) -> x2j bf16, DMA-transpose -> x2t
  V   vaug [S, H, 65] bf16 (65th column = ones; softmax denominator for free)
  P3  per head-pair pr: QT/KT = w^T @ x2t (SBUF-resident, bf16),
      per j: scoresT = K @ Q^T (row-packed 64x2 heads), exp via ACT with
      per-partition mask bias, attnT[65, S] += [V|1]^T @ expT.
      Denominators: PE-transpose the two [1,S] rows -> [128,16], one cheap
      DVE reciprocal, DMA round-trip to broadcast [64,S], normalize on evac.
  P4  out-proj O = catT^T @ wo (+ residual x) -> out1 seq-major (f32)
  P5  LN2 (bn_stats) -> x2b bf16, DMA-transpose -> x2t (reused)
  P6  HT = w1^T @ x2bt, relu (+b1) -> ht bf16
  P7  out2 = ht^T @ w2 (+ out1 residual) -> y
"""
import sys

sys.path.insert(0, "/opt/trn_rl_repo")

import numpy as np
import ml_dtypes

import concourse.bass as bass
import concourse.mybir as mybir
from concourse import bacc
from concourse.tile import TileContext
from concourse.bass_utils import run_bass_kernel_spmd
from concourse.masks import make_identity

P = 128
S = 1024
D = 1024
H = 16
DK = 64
F = 2048
NT = S // P   # seq tiles
KD = D // P   # feature k-tiles
KF = F // P   # ff k-tiles
EPS = 1e-6

F32 = mybir.dt.float32
BF16 = mybir.dt.bfloat16
Alu = mybir.AluOpType
Act = mybir.ActivationFunctionType
BF16NP = ml_dtypes.bfloat16

# smalls layout (columns of a [128, 56] tensor)
C_MU, C_R1, C_MB, C_BQ, C_BK, C_B1 = 0, 8, 16, 24, 32, 40  # b1 gets 16 cols

_CACHE = {}
LAST_RESULT = None


def _build(flags):
    has_bv, has_bo, has_b2 = flags
    nc = bacc.Bacc()

    x_d = nc.dram_tensor("x", [NT, P, D], F32, kind="ExternalInput")
    sm_d = nc.dram_tensor("smalls", [P, 56], F32, kind="ExternalInput")
    wq_d = nc.dram_tensor("wq", [KD, P, D], BF16, kind="ExternalInput")
    wk_d = nc.dram_tensor("wk", [KD, P, D], BF16, kind="ExternalInput")
    wv_d = nc.dram_tensor("wv", [KD, P, D], BF16, kind="ExternalInput")
    wo_d = nc.dram_tensor("wo", [KD, P, D], BF16, kind="ExternalInput")
    w1_d = nc.dram_tensor("w1", [KD, P, F], BF16, kind="ExternalInput")
    w2_d = nc.dram_tensor("w2", [KF, P, D], BF16, kind="ExternalInput")
    if has_bv:
        bv_d = nc.dram_tensor("bv", [1, D], F32, kind="ExternalInput")
    if has_bo:
        bo_d = nc.dram_tensor("bo", [1, D], F32, kind="ExternalInput")
    if has_b2:
        b2_d = nc.dram_tensor("b2", [1, D], F32, kind="ExternalInput")
    y_d = nc.dram_tensor("y", [NT, P, D], F32, kind="ExternalOutput")

    rd_d = nc.dram_tensor("rd_scratch", [H, S], F32)
    dn_d = nc.dram_tensor("dn_scratch", [H, S], F32)

    with TileContext(nc) as tc:
        with tc.tile_pool(name="const", bufs=1) as constp, \
             tc.tile_pool(name="big", bufs=1) as bigp:
            smalls = constp.tile([P, 56], F32)
            nc.sync.dma_start(out=smalls, in_=sm_d[:, :])
            ident = constp.tile([P, P], BF16)
            make_identity(nc, ident)

            def bias_bcast(dram_row):
                src_ap = dram_row[0:1, :]
                bc_ap = bass.AP(tensor=src_ap.tensor, offset=src_ap.offset,
                                ap=[[0, P]] + list(src_ap.ap)[1:])
                bc = constp.tile([P, D], F32)
                nc.sync.dma_start(out=bc, in_=bc_ap)
                return bc

            bvB = bias_bcast(bv_d) if has_bv else None
            boB = bias_bcast(bo_d) if has_bo else None
            b2B = bias_bcast(b2_d) if has_b2 else None

            out1 = bigp.tile([P, NT, D], F32, tag="out1")
            x2t = bigp.tile([P, KD, S], BF16, tag="x2t")

            attl_cm = tc.tile_pool(name="attl", bufs=1)
            attl = attl_cm.__enter__()
            vaug = attl.tile([P, NT, H, 65], BF16, tag="vaug")
            cat = attl.tile([P, KD, S], BF16, tag="cat")

            # ---------------- P1: LN1 apply + PE transpose ----------------
            with tc.tile_pool(name="p1", bufs=3) as p1, \
                 tc.tile_pool(name="pstr", bufs=2, space="PSUM") as pstr:
                for j in range(NT):
                    xj = p1.tile([P, D], F32, tag="xj")
                    nc.sync.dma_start(out=xj, in_=x_d[j])
                    x2j = p1.tile([P, D], BF16, tag="x2j")
                    nc.vector.tensor_scalar(
                        out=x2j, in0=xj,
                        scalar1=smalls[:, C_MU + j:C_MU + j + 1],
                        scalar2=smalls[:, C_R1 + j:C_R1 + j + 1],
                        op0=Alu.subtract, op1=Alu.mult)
                    for a in range(2):
                        ps = pstr.tile([P, 512], BF16, tag="tr")
                        for q in range(4):
                            i = 4 * a + q
                            nc.tensor.transpose(
                                ps[:, q * P:(q + 1) * P],
                                x2j[:, i * P:(i + 1) * P], ident)
                        nc.vector.tensor_copy(
                            out=x2t[:, 4 * a:4 * a + 4, j * P:(j + 1) * P],
                            in_=ps.rearrange("p (a b) -> p a b", b=P))

            # ---------------- V: vaug [S, H, 65] ----------------
            with tc.tile_pool(name="wvp", bufs=1) as wvp, \
                 tc.tile_pool(name="psv", bufs=4, space="PSUM") as psv:
                ones16 = constp.tile([P, H], F32)
                nc.vector.memset(ones16, 1.0)
                for i in range(NT):
                    nc.vector.tensor_copy(
                        out=vaug[:, i, :, 64:65],
                        in_=ones16.rearrange("p (h o) -> p h o", o=1))
                wv_sl = []
                for k in range(KD):
                    t = wvp.tile([P, D], BF16, tag=f"wv{k}")
                    nc.scalar.dma_start(out=t, in_=wv_d[k])
                    wv_sl.append(t)
                for n in range(2):
                    for i in range(NT):
                        ps = psv.tile([P, 512], F32, tag="mm")
                        for k in range(KD):
                            nc.tensor.matmul(
                                ps, x2t[:, k, i * P:(i + 1) * P],
                                wv_sl[k][:, n * 512:(n + 1) * 512],
                                start=(k == 0), stop=(k == KD - 1))
                        dst = vaug[:, i, 8 * n:8 * n + 8, 0:64]
                        if has_bv:
                            nc.vector.tensor_add(
                                out=dst, in0=ps.rearrange("p (h c) -> p h c", c=64),
                                in1=bvB[:, n * 512:(n + 1) * 512].rearrange(
                                    "p (h c) -> p h c", c=64))
                        else:
                            nc.vector.tensor_copy(
                                out=dst, in_=ps.rearrange("p (h c) -> p h c", c=64))

            # -------- P3: per head-pair QT/KT + attention --------
            wopre_cm = tc.tile_pool(name="wopre", bufs=1)
            wopre = wopre_cm.__enter__()
            xr_cm = tc.tile_pool(name="xr", bufs=1)
            xrp = xr_cm.__enter__()
            wo_tiles = {}
            xm_sl = []
            with tc.tile_pool(name="wp", bufs=2) as wp, \
                 tc.tile_pool(name="qk", bufs=2) as qkp, \
                 tc.tile_pool(name="ep", bufs=3) as ep, \
                 tc.tile_pool(name="dn", bufs=2) as dnp, \
                 tc.tile_pool(name="rr", bufs=2) as rrp, \
                 tc.tile_pool(name="rb", bufs=2) as rbp, \
                 tc.tile_pool(name="sc", bufs=2, space="PSUM") as scp, \
                 tc.tile_pool(name="at", bufs=2, space="PSUM") as atp:
                for pr in range(KD):
                    hA, hB = 2 * pr, 2 * pr + 1
                    # --- QT / KT for this head pair (SBUF-resident bf16) ---
                    wq8 = wp.tile([P, KD, P], BF16, tag="wq8")
                    nc.scalar.dma_start(
                        out=wq8,
                        in_=wq_d[:, :, pr * P:(pr + 1) * P].rearrange("k p m -> p k m"))
                    wk8 = wp.tile([P, KD, P], BF16, tag="wk8")
                    nc.scalar.dma_start(
                        out=wk8,
                        in_=wk_d[:, :, pr * P:(pr + 1) * P].rearrange("k p m -> p k m"))
                    qps = atp.tile([P, S], F32, tag="at")
                    for n in range(2):
                        for k in range(KD):
                            nc.tensor.matmul(
                                qps[:, n * 512:(n + 1) * 512], wq8[:, k, :],
                                x2t[:, k, n * 512:(n + 1) * 512],
                                start=(k == 0), stop=(k == KD - 1))
                    qtp = qkp.tile([P, S], BF16, tag="qt")
                    nc.vector.tensor_scalar(
                        out=qtp, in0=qps,
                        scalar1=smalls[:, C_BQ + pr:C_BQ + pr + 1], scalar2=None,
                        op0=Alu.add)
                    kps = atp.tile([P, S], F32, tag="at")
                    for n in range(2):
                        for k in range(KD):
                            nc.tensor.matmul(
                                kps[:, n * 512:(n + 1) * 512], wk8[:, k, :],
                                x2t[:, k, n * 512:(n + 1) * 512],
                                start=(k == 0), stop=(k == KD - 1))
                    ktp = qkp.tile([P, S], BF16, tag="kt")
                    nc.vector.tensor_scalar(
                        out=ktp, in0=kps,
                        scalar1=smalls[:, C_BK + pr:C_BK + pr + 1], scalar2=None,
                        op0=Alu.add)

                    # --- attention over key tiles j (software-pipelined:
                    #     scores/exp for j+1 are emitted before attn for j so
                    #     the in-order PE never waits on the scalar engine) ---
                    aA = atp.tile([P, S], F32, tag="at")
                    aB = atp.tile([P, S], F32, tag="at")
                    e_tiles = {}

                    def scores_exp(j):
                        scA = scp.tile([P, S], F32, tag="sc", name="scA")
                        scB = scp.tile([P, S], F32, tag="sc", name="scB")
                        for n in range(2):
                            nc.tensor.matmul(
                                scA[:, n * 512:(n + 1) * 512],
                                ktp[0:64, j * P:(j + 1) * P],
                                qtp[0:64, n * 512:(n + 1) * 512],
                                start=True, stop=True, tile_position=(0, 0))
                            nc.tensor.matmul(
                                scB[:, n * 512:(n + 1) * 512],
                                ktp[64:P, j * P:(j + 1) * P],
                                qtp[64:P, n * 512:(n + 1) * 512],
                                start=True, stop=True, tile_position=(64, 0))
                        eA = ep.tile([P, S], BF16, tag="exp", bufs=4, name="eA")
                        nc.scalar.activation(
                            out=eA, in_=scA, func=Act.Exp,
                            bias=smalls[:, C_MB + j:C_MB + j + 1], scale=0.125)
                        eB = ep.tile([P, S], BF16, tag="exp", bufs=4, name="eB")
                        nc.scalar.activation(
                            out=eB, in_=scB, func=Act.Exp,
                            bias=smalls[:, C_MB + j:C_MB + j + 1], scale=0.125)
                        e_tiles[j] = (eA, eB)

                    def attn(j):
                        eA, eB = e_tiles.pop(j)
                        for n in range(2):
                            nc.tensor.matmul(
                                aA[0:65, n * 512:(n + 1) * 512],
                                vaug[:, j, hA, :],
                                eA[:, n * 512:(n + 1) * 512],
                                start=(j == 0), stop=(j == NT - 1))
                            nc.tensor.matmul(
                                aB[0:65, n * 512:(n + 1) * 512],
                                vaug[:, j, hB, :],
                                eB[:, n * 512:(n + 1) * 512],
                                start=(j == 0), stop=(j == NT - 1))

                    scores_exp(0)
                    for j in range(1, NT):
                        scores_exp(j)
                        attn(j - 1)
                    attn(NT - 1)

                    # evacuate attn psum immediately (frees the at-pool banks
                    # for the next head pair's QT/KT)
                    cpA = dnp.tile([65, S], F32, tag="cpA")
                    nc.vector.tensor_copy(out=cpA, in_=aA[0:65, :])
                    cpB = dnp.tile([65, S], F32, tag="cpB")
                    nc.vector.tensor_copy(out=cpB, in_=aB[0:65, :])

                    # --- denominators: row 64 -> DRAM -> strided gather
                    #     [p, a] -> one cheap recip -> DRAM -> bcast ---
                    nc.sync.dma_start(out=dn_d[hA:hA + 1, :], in_=cpA[64:65, :])
                    nc.sync.dma_start(out=dn_d[hB:hB + 1, :], in_=cpB[64:65, :])
                    # contiguous gather: dn_d[h] viewed [NT, 128] with key
                    # tile on partitions — 512B/partition chunks
                    rrs = rrp.tile([2 * NT, P], F32, tag="rrs")
                    for hb in (0, 1):
                        g_ap = bass.AP(tensor=dn_d[0:1, :].tensor,
                                       offset=(2 * pr + hb) * S,
                                       ap=[[P, NT], [1, P]])
                        nc.sync.dma_start(out=rrs[hb * NT:(hb + 1) * NT, :], in_=g_ap)
                    rrinv = rrp.tile([2 * NT, P], F32, tag="rr")
                    nc.vector.reciprocal(out=rrinv, in_=rrs)

                    def bcast_recip(hb):
                        h = 2 * pr + hb
                        dst = bass.AP(tensor=rd_d[0:1, :].tensor, offset=h * S,
                                      ap=[[P, NT], [1, P]])
                        nc.sync.dma_start(out=dst,
                                          in_=rrinv[hb * NT:(hb + 1) * NT, :])
                        s_ap = rd_d[h:h + 1, :]
                        bc_ap = bass.AP(tensor=s_ap.tensor, offset=s_ap.offset,
                                        ap=[[0, 64]] + list(s_ap.ap)[1:])
                        rbt = rbp.tile([64, S], F32, tag="rb")
                        nc.sync.dma_start(out=rbt, in_=bc_ap)
                        return rbt

                    rbA = bcast_recip(0)
                    nc.vector.tensor_mul(
                        out=cat[0:64, pr, :], in0=cpA[0:64, :], in1=rbA)
                    rbB = bcast_recip(1)
                    stg = dnp.tile([64, S], BF16, tag="stg")
                    nc.vector.tensor_mul(out=stg, in0=cpB[0:64, :], in1=rbB)
                    nc.sync.dma_start(out=cat[64:P, pr, :], in_=stg)

                    # prefetch wo and residual x while attention runs
                    if pr < 4:
                        for k in (2 * pr, 2 * pr + 1):
                            t = wopre.tile([P, D], BF16, tag=f"wo{k}", name=f"wo{k}")
                            nc.scalar.dma_start(out=t, in_=wo_d[k])
                            wo_tiles[k] = t
                    else:
                        for m in (2 * (pr - 4), 2 * (pr - 4) + 1):
                            xm = xrp.tile([P, D], F32, tag=f"xm{m}", name=f"xm{m}")
                            nc.sync.dma_start(out=xm, in_=x_d[m])
                            xm_sl.append(xm)

            # ---------------- P4 + P5 merged m-loop ----------------
            with tc.tile_pool(name="p45", bufs=3) as p45, \
                 tc.tile_pool(name="pstr2", bufs=2, space="PSUM") as pstr2, \
                 tc.tile_pool(name="pm", bufs=4, space="PSUM") as pm:
                for m in range(NT):
                    xm = xm_sl[m]
                    for n in range(2):
                        ps = pm.tile([P, 512], F32, tag="mm")
                        # rotate k-order so the last matmul of tile m only
                        # needs cat[:, m] — absorbs the attention tail latency
                        for ki in range(KD):
                            k = (m + 1 + ki) % KD
                            nc.tensor.matmul(
                                ps, cat[:, k, m * P:(m + 1) * P],
                                wo_tiles[k][:, n * 512:(n + 1) * 512],
                                start=(ki == 0), stop=(ki == KD - 1))
                        dst = out1[:, m, n * 512:(n + 1) * 512]
                        nc.vector.tensor_add(
                            out=dst, in0=ps, in1=xm[:, n * 512:(n + 1) * 512])
                        if has_bo:
                            nc.vector.tensor_add(
                                out=dst, in0=dst, in1=boB[:, n * 512:(n + 1) * 512])
                    # LN2 for this m
                    row = out1[:, m, :]
                    st = p45.tile([P, 2, 6], F32, tag="st")
                    nc.vector.bn_stats(
                        out=st[:, 0, :],
                        in_=row.rearrange("p (a b) -> p a b", b=512)[:, 0, :])
                    nc.vector.bn_stats(
                        out=st[:, 1, :],
                        in_=row.rearrange("p (a b) -> p a b", b=512)[:, 1, :])
                    mv = p45.tile([P, 2], F32, tag="mv")
                    nc.vector.bn_aggr(out=mv, in_=st)
                    sd = p45.tile([P, 1], F32, tag="sd")
                    nc.scalar.activation(
                        out=sd, in_=mv[:, 1:2], func=Act.Sqrt,
                        scale=float(S) / float(S - 1))
                    sde = p45.tile([P, 1], F32, tag="sde")
                    nc.vector.tensor_scalar(
                        out=sde, in0=sd, scalar1=EPS, scalar2=None, op0=Alu.add)
                    r2 = p45.tile([P, 1], F32, tag="r2")
                    nc.vector.reciprocal(out=r2, in_=sde)
                    x2b = p45.tile([P, D], BF16, tag="x2b")
                    nc.vector.tensor_scalar(
                        out=x2b, in0=row, scalar1=mv[:, 0:1], scalar2=r2,
                        op0=Alu.subtract, op1=Alu.mult)
                    for a in range(2):
                        ps2 = pstr2.tile([P, 512], BF16, tag="tr")
                        for q in range(4):
                            i = 4 * a + q
                            nc.tensor.transpose(
                                ps2[:, q * P:(q + 1) * P],
                                x2b[:, i * P:(i + 1) * P], ident)
                        nc.vector.tensor_copy(
                            out=x2t[:, 4 * a:4 * a + 4, m * P:(m + 1) * P],
                            in_=ps2.rearrange("p (a b) -> p a b", b=P))
            xr_cm.__exit__(None, None, None)
            wopre_cm.__exit__(None, None, None)
            attl_cm.__exit__(None, None, None)

            # ---------------- P6 + P7 interleaved by seq half ----------------
            with tc.tile_pool(name="ffn", bufs=1) as ffnp, \
                 tc.tile_pool(name="w1p", bufs=3) as w1p, \
                 tc.tile_pool(name="w2w", bufs=1) as w2w, \
                 tc.tile_pool(name="yst", bufs=3) as yst, \
                 tc.tile_pool(name="pf", bufs=4, space="PSUM") as pf:
                ht_lo = ffnp.tile([P, KD, S], BF16, tag="ht_lo")
                ht_hi = ffnp.tile([P, KD, S], BF16, tag="ht_hi")
                ht = [ht_lo, ht_hi]
                w2_sl = []
                for kf in range(KF):
                    t = w2w.tile([P, D], BF16, tag=f"w2_{kf}")
                    nc.sync.dma_start(out=t, in_=w2_d[kf])
                    w2_sl.append(t)
                for n in range(2):
                    for f in range(KF):
                        t8 = w1p.tile([P, KD, P], BF16, tag="w18")
                        nc.scalar.dma_start(
                            out=t8,
                            in_=w1_d[:, :, f * P:(f + 1) * P].rearrange("k p m -> p k m"))
                        ps = pf.tile([P, 512], F32, tag="mm")
                        for k in range(KD):
                            nc.tensor.matmul(
                                ps, t8[:, k, :],
                                x2t[:, k, n * 512:(n + 1) * 512],
                                start=(k == 0), stop=(k == KD - 1))
                        nc.vector.tensor_scalar(
                            out=ht[f // KD][:, f % KD, n * 512:(n + 1) * 512],
                            in0=ps, scalar1=smalls[:, C_B1 + f:C_B1 + f + 1],
                            scalar2=0.0, op0=Alu.add, op1=Alu.max)
                    for m in range(4 * n, 4 * n + 4):
                        for nn in range(2):
                            ps = pf.tile([P, 512], F32, tag="mm")
                            for kf in range(KF):
                                nc.tensor.matmul(
                                    ps, ht[kf // KD][:, kf % KD, m * P:(m + 1) * P],
                                    w2_sl[kf][:, nn * 512:(nn + 1) * 512],
                                    start=(kf == 0), stop=(kf == KF - 1))
                            yt = yst.tile([P, 512], F32, tag="yt")
                            nc.vector.tensor_add(
                                out=yt, in0=ps,
                                in1=out1[:, m, nn * 512:(nn + 1) * 512])
                            if has_b2:
                                nc.vector.tensor_add(
                                    out=yt, in0=yt,
                                    in1=b2B[:, nn * 512:(nn + 1) * 512])
                            nc.sync.dma_start(
                                out=y_d[m, :, nn * 512:(nn + 1) * 512], in_=yt)

    nc.compile()
    return nc


def _col_tiles(v, ncols):
    """[N] -> [128, ncols] with element 128*j + i at [i, j]."""
    return np.ascontiguousarray(v.reshape(ncols, P).T)


def kernel(x, mask, n1_a, n1_b, n2_a, n2_b, wq, bq, wk, bk, wv, bv,
           wo, bo, w1, b1, w2, b2):
    global LAST_RESULT
    x = np.asarray(x, dtype=np.float32)
    mask = np.asarray(mask)
    f32 = lambda a: np.asarray(a, dtype=np.float32)
    n1_a, n1_b, n2_a, n2_b = map(f32, (n1_a, n1_b, n2_a, n2_b))
    wq, bq, wk, bk, wv, bv = map(f32, (wq, bq, wk, bk, wv, bv))
    wo, bo, w1, b1, w2, b2 = map(f32, (wo, bo, w1, b1, w2, b2))
    B = x.shape[0]
    assert x.shape == (B, S, D) and B == 8

    # fold LN affine params into following matmuls
    wq_e = n1_a[:, None] * wq
    wk_e = n1_a[:, None] * wk
    wv_e = n1_a[:, None] * wv
    bq_e = n1_b @ wq + bq
    bk_e = n1_b @ wk + bk
    bv_e = n1_b @ wv + bv
    w1_e = n2_a[:, None] * w1
    b1_e = n2_b @ w1 + b1

    # LN1 stats on host (input-only reduction)
    mu1 = x.mean(axis=-1, dtype=np.float32)                # [B, S]
    sd1 = x.std(axis=-1, ddof=1, dtype=np.float32)         # [B, S]
    r1 = 1.0 / (sd1 + EPS)
    maskb = np.where(mask[:, 0, :] == 0, np.float32(-1e5), np.float32(0.0))

    flags = (bool(bv_e.any()), bool(bo.any()), bool(b2.any()))
    if flags not in _CACHE:
        _CACHE[flags] = _build(flags)
    nc = _CACHE[flags]

    bf = lambda a: np.ascontiguousarray(a).astype(BF16NP)
    wq_t = bf(wq_e.reshape(KD, P, D))
    wk_t = bf(wk_e.reshape(KD, P, D))
    wv_t = bf(wv_e.reshape(KD, P, D))
    wo_t = bf(wo.reshape(KD, P, D))
    w1_t = bf(w1_e.reshape(KD, P, F))
    w2_t = bf(w2.reshape(KF, P, D))
    bq_c = _col_tiles(bq_e, KD)
    bk_c = _col_tiles(bk_e, KD)
    b1_c = _col_tiles(b1_e, KF)

    in_maps = []
    for b in range(B):
        smalls = np.zeros((P, 56), dtype=np.float32)
        smalls[:, C_MU:C_MU + NT] = _col_tiles(mu1[b], NT)
        smalls[:, C_R1:C_R1 + NT] = _col_tiles(r1[b], NT)
        smalls[:, C_MB:C_MB + NT] = _col_tiles(maskb[b], NT)
        smalls[:, C_BQ:C_BQ + KD] = bq_c
        smalls[:, C_BK:C_BK + KD] = bk_c
        smalls[:, C_B1:C_B1 + KF] = b1_c
        m = {
            "x": np.ascontiguousarray(x[b].reshape(NT, P, D)),
            "smalls": smalls,
            "wq": wq_t, "wk": wk_t, "wv": wv_t, "wo": wo_t,
            "w1": w1_t, "w2": w2_t,
        }
        if flags[0]:
            m["bv"] = bv_e.reshape(1, D).astype(np.float32)
        if flags[1]:
            m["bo"] = bo.reshape(1, D).astype(np.float32)
        if flags[2]:
            m["b2"] = b2.reshape(1, D).astype(np.float32)
        in_maps.append(m)

    res = run_bass_kernel_spmd(nc, in_maps, core_ids=list(range(8)))
    LAST_RESULT = res
    out = np.stack([res.results[b]["y"].reshape(S, D) for b in range(B)])
    return out
